# revision 1
# baseline (speedup 1.0000x reference)
"""Multi-head causal attention Bass kernel for Trainium2, 8-core SPMD.

Problem: B=2, S=2048, D=1024, H=16, DH=64.
  q = x @ Wq; k = x @ Wk; v = x @ Wv  (per head h: 64-wide column slices)
  out = softmax(causal(q k^T / 8)) v

Sharding: core c -> batch b = c // 4, head group g = c % 4 (heads 4g..4g+3).
Each core gets x[b]^T (transposed on host) and 256-wide W column slices,
computes 4 heads over the full sequence, returns y [2, 128, 2048] =
ctx^T stacked per head pair. Host reassembles/transposes.

Per-core layout (matmul operands float32r = full-rate, ~tf32 accuracy):
  xT_ch[ch] [128, 8*512]   ch = s-chunk; k-chunk kk at cols [512*kk, ...)
  w*_all    [128, 8*256]   k-chunk kk at cols [256*kk, ...)
  QT/KT     per (m, ch) tiles [128, 512] (rows = W cols j; head h at
                                          tile h//2, partitions (h%2)*64)
  V4[q]     [128, 4*260]   s-tiles 4q..4q+3; within a 260-block: per head
                           64 V cols + 1 ones col (softmax denominator row)
  scores^T per (head, i-chunk of 512) in j-groups of 2 tiles:
      psum [128, 1024] -> exp via ACT (scale=1/8) -> SBUF f32r
      causal diagonal blocks masked via gpsimd affine_select (fill 0)
  ctx^T[e, i] accumulated in psum [65, 512] over j-tiles; row 64 = denom l[i]
  normalize: l -> SBUF -> DVE recip_approx -> gpsimd partition_broadcast ->
  DVE mul -> ctx_sb -> DMA out
"""

import sys

import numpy as np

try:
    import concourse.bass as bass  # noqa: F401
except ImportError:
    for _p in ("/opt/trn_rl_repo", "/root/.axon_site/_ro/trn_rl_repo"):
        if _p not in sys.path:
            sys.path.insert(0, _p)
    import concourse.bass as bass  # noqa: F401

from concourse import bacc
import concourse.mybir as mybir
import concourse.tile as tile

F32 = mybir.dt.float32
F32R = mybir.dt.float32r

S = 2048          # sequence length
D = 1024          # model dim (contraction for projections)
HPC = 4           # heads per core
DH = 64           # head dim
NK = D // 128     # 8 contraction chunks
NST = S // 128    # 16 sequence tiles of 128
NCH = S // 512    # 4 s-chunks of 512
VW = HPC * (DH + 1)   # 260 cols per V s-tile


def build_kernel(loop_n=0):
    nc = bacc.Bacc("TRN2", target_bir_lowering=False, debug=True)

    xT = nc.dram_tensor("xT", [D, S], F32R, kind="ExternalInput")
    wq = nc.dram_tensor("wq", [D, HPC * DH], F32R, kind="ExternalInput")
    wk = nc.dram_tensor("wk", [D, HPC * DH], F32R, kind="ExternalInput")
    wv = nc.dram_tensor("wv", [D, HPC * DH], F32R, kind="ExternalInput")
    ones_in = nc.dram_tensor("ones_in", [128, 4, HPC, 1], F32R,
                             kind="ExternalInput")
    y = nc.dram_tensor("y", [2, 128, S], F32, kind="ExternalOutput")

    with tile.TileContext(nc) as tc:
        from contextlib import ExitStack
        stk = ExitStack()
        loop = stk.enter_context(tc.For_i(0, loop_n, 1)) if loop_n else None
        with stk, (
            tc.tile_pool(name="persist", bufs=1)
        ) as pers, (
            tc.tile_pool(name="proj_ps", bufs=2, space="PSUM")
        ) as proj_ps, (
            tc.tile_pool(name="score_ps", bufs=2, space="PSUM")
        ) as score_ps, (
            tc.tile_pool(name="ctx_ps", bufs=2, space="PSUM")
        ) as ctx_ps_pool, (
            tc.tile_pool(name="esb", bufs=4)
        ) as esb_pool, (
            tc.tile_pool(name="norm", bufs=2)
        ) as norm_pool:
            # ---- persistent SBUF tiles -------------------------------------
            xT_ch = [
                pers.tile([128, NK * 512], F32R, tag=f"xTc{ch}", name=f"xTc{ch}")
                for ch in range(NCH)
            ]
            w_all = {
                wname: pers.tile([128, NK * HPC * DH], F32R, name=f"w_{wname}")
                for wname in ("q", "k", "v")
            }
            QT_sb = [
                [pers.tile([128, 512], F32R, tag=f"QT{m}c{ch}",
                           name=f"QT{m}c{ch}") for ch in range(NCH)]
                for m in range(2)
            ]
            KT_sb = [
                [pers.tile([128, 512], F32R, tag=f"KT{m}c{ch}",
                           name=f"KT{m}c{ch}") for ch in range(NCH)]
                for m in range(2)
            ]
            V4 = [
                pers.tile([128, 4 * VW], F32R, tag=f"V4_{q}", name=f"V4_{q}")
                for q in range(4)
            ]
            ctx_sb = [
                pers.tile([128, S], F32, tag=f"ctx{p}", name=f"ctx{p}")
                for p in range(2)
            ]

            def xs(ch, kk):      # xT chunk ch, k-chunk kk -> [128, 512]
                return xT_ch[ch][:, 512 * kk:512 * (kk + 1)]

            def ws(wname, kk):   # w k-chunk [128, 256]
                return w_all[wname][:, HPC * DH * kk:HPC * DH * (kk + 1)]

            def vs(t):           # V s-tile t -> [128, 260]
                return V4[t // 4][:, VW * (t % 4):VW * (t % 4 + 1)]

            # ---- input DMAs ------------------------------------------------
            # W first (small, needed by every projection) on the SP HWDGE ring;
            # xT s-chunks on the ACT HWDGE ring so they stream in parallel.
            for wname, wdram in (("q", wq), ("k", wk), ("v", wv)):
                nc.sync.dma_start(
                    out=w_all[wname].rearrange("p (k e) -> p k e", k=NK),
                    in_=wdram.rearrange("(k p) e -> p k e", k=NK),
                )
            xTr = xT.rearrange("(k p) (c s) -> p c k s", k=NK, c=NCH)
            for ch in range(NCH):
                nc.scalar.dma_start(
                    out=xT_ch[ch].rearrange("p (k s) -> p k s", k=NK),
                    in_=xTr[:, ch],
                )
            # ones columns of V (denominator rows): one strided DMA per V4
            for q in range(4):
                nc.sync.dma_start(
                    out=V4[q].rearrange("p (t h c) -> p t h c", t=4, h=HPC)[
                        :, :, :, DH:DH + 1
                    ],
                    in_=ones_in[:],
                )

            # ---- projections (emitted per s-chunk, interleaved with
            # attention: attention i-chunk c needs only chunks <= c) --------
            def emit_proj_chunk(ch):
                # Q^T/K^T: out[j, s] = sum_d W[d, j] * xT[d, s]
                for m in range(2):
                    for wname, dest in (("q", QT_sb), ("k", KT_sb)):
                        ps = proj_ps.tile([128, 512], F32, tag="proj", name="ps_qk")
                        for kk in range(NK):
                            nc.tensor.matmul(
                                ps[:],
                                ws(wname, kk)[:, 128 * m:128 * (m + 1)],
                                xs(ch, kk),
                                start=(kk == 0),
                                stop=(kk == NK - 1),
                            )
                        nc.vector.tensor_copy(dest[m][ch][:], ps[:])
                # V: out[s, e] = sum_d xT[d, s] * Wv[d, e]
                for t in range(4 * ch, 4 * ch + 4):
                    ps = proj_ps.tile([128, HPC * DH], F32, tag="proj", name="ps_v")
                    for kk in range(NK):
                        nc.tensor.matmul(
                            ps[:],
                            xs(t // 4, kk)[:, 128 * (t % 4):128 * (t % 4 + 1)],
                            ws("v", kk),
                            start=(kk == 0),
                            stop=(kk == NK - 1),
                        )
                    nc.vector.tensor_copy(
                        vs(t).rearrange("p (h c) -> p h c", h=HPC)[:, :, 0:DH],
                        ps.rearrange("p (h c) -> p h c", h=HPC),
                    )

            # ---- attention -----------------------------------------
            # Head-PAIR packed scores: for pair p = h//2, one psum
            # [128, 1024] holds head A (cols 0:512) and head B
            # (cols 512:1024) scores^T for ONE j-tile, computed by two
            # row-strip-packed K=64 matmuls that run concurrently on the
            # PE sub-arrays. One exp covers both heads. ctx accumulates
            # per head in its own [65, 512] psum (ones row = denom).
            e_sbs = {}

            def emit_scores(key):
                pair, c, jt = key
                m = pair
                ps = score_ps.tile([128, 1024], F32, tag="score",
                                   name="s_ps")
                for half in range(2):
                    off = half * 64
                    nc.tensor.matmul(
                        ps[:, 512 * half:512 * (half + 1)],
                        KT_sb[m][jt // 4][off:off + 64,
                                          128 * (jt % 4):128 * (jt % 4 + 1)],
                        QT_sb[m][c][off:off + 64, :],
                        start=True,
                        stop=True,
                        tile_position=(off, 0),
                    )
                e = esb_pool.tile([128, 1024], F32R, tag="esb", name="e_sb")
                nc.scalar.activation(
                    out=e[:], in_=ps[:],
                    func=mybir.ActivationFunctionType.Exp, scale=0.125,
                )
                if jt >= 4 * c:
                    # diagonal j-tile: keep where di - dj - o >= 0
                    o = (jt - 4 * c) * 128
                    e3 = e.rearrange("p (h i) -> p h i", h=2)
                    nc.gpsimd.affine_select(
                        e3,
                        e3,
                        pattern=[[0, 2], [1, 512]],
                        compare_op=mybir.AluOpType.is_ge,
                        fill=0.0,
                        base=-o,
                        channel_multiplier=-1,
                    )
                e_sbs[key] = e

            def finish_ctx(ctx_psum, h, c):
                pair, off = h // 2, (h % 2) * 64
                lrow = norm_pool.tile([1, 512], F32, tag="lrow", name="lrow")
                nc.vector.tensor_copy(lrow[:], ctx_psum[64:65, :])
                recip = norm_pool.tile([1, 512], F32, tag="recip",
                                       name="recip")
                nc.vector.reciprocal_approx_fast(out=recip[:], in_=lrow[:])
                bc = norm_pool.tile([64, 512], F32, tag="bc", name="bc")
                nc.gpsimd.partition_broadcast(bc[:], recip[:])
                nc.vector.tensor_mul(
                    ctx_sb[pair][off:off + 64, 512 * c:512 * (c + 1)],
                    ctx_psum[0:64, :],
                    bc[:],
                )
                if h % 2 == 1:
                    nc.sync.dma_start(
                        out=y[pair, :, 512 * c:512 * (c + 1)],
                        in_=ctx_sb[pair][:, 512 * c:512 * (c + 1)],
                    )

            # finish chains are emitted one pair LATE so their gpsimd
            # partition_broadcast queues behind the next pair's first causal
            # masks instead of ahead of them (gpsimd executes in FIFO order)
            pending_finish = []

            def flush_finish():
                while pending_finish:
                    args = pending_finish.pop(0)
                    finish_ctx(*args)

            for c in range(NCH):
                emit_proj_chunk(c)
                njt = 4 * (c + 1)
                for pair in range(2):
                    keys = [(pair, c, jt) for jt in range(njt)]
                    emit_scores(keys[0])
                    flush_finish()
                    ctxA = ctx_ps_pool.tile([65, 512], F32, tag="ctx",
                                            name="ctx_psA")
                    ctxB = ctx_ps_pool.tile([65, 512], F32, tag="ctx",
                                            name="ctx_psB")
                    for idx, key in enumerate(keys):
                        if idx + 1 < len(keys):
                            emit_scores(keys[idx + 1])
                        _, _, jt = key
                        e = e_sbs.pop(key)
                        for half, cps in ((0, ctxA), (1, ctxB)):
                            h = 2 * pair + half
                            nc.tensor.matmul(
                                cps[:],
                                vs(jt)[:, (DH + 1) * h:(DH + 1) * (h + 1)],
                                e[:, 512 * half:512 * (half + 1)],
                                start=(idx == 0),
                                stop=(idx == njt - 1),
                            )
                    pending_finish.append((ctxA, 2 * pair, c))
                    pending_finish.append((ctxB, 2 * pair + 1, c))
            flush_finish()
    nc.compile()
    return nc


_CACHED = None


def get_nc():
    global _CACHED
    if _CACHED is None:
        _CACHED = build_kernel()
    return _CACHED


def shard_inputs(x, W_query, W_key, W_value):
    """Full inputs -> per-core input maps."""
    in_maps = []
    ones = np.ones((128, 4, HPC, 1), np.float32)
    # one transpose per batch, shared by the 4 cores of that batch
    xT_by_batch = [np.ascontiguousarray(x[b].T) for b in range(2)]
    for core in range(8):
        b, g = core // 4, core % 4
        sl = slice(256 * g, 256 * (g + 1))
        in_maps.append({
            "xT": xT_by_batch[b],
            "wq": np.ascontiguousarray(W_query[:, sl]),
            "wk": np.ascontiguousarray(W_key[:, sl]),
            "wv": np.ascontiguousarray(W_value[:, sl]),
            "ones_in": ones,
        })
    return in_maps


def assemble_output(results):
    """Per-core y [2, 128, S] -> full [2, S, 1024]."""
    out = np.empty((2, S, 1024), np.float32)
    for core in range(8):
        b, g = core // 4, core % 4
        yv = results[core]["y"]  # [2, 128, S]
        blk = yv.reshape(2, 2, 64, S).transpose(3, 0, 1, 2).reshape(S, 256)
        out[b, :, 256 * g:256 * (g + 1)] = blk
    return out


def kernel(x, W_query, W_key, W_value):
    """Full inputs in, full output out; 8-core SPMD underneath."""
    from concourse.bass_utils import run_bass_kernel_spmd

    x = np.ascontiguousarray(np.asarray(x, dtype=np.float32))
    W_query = np.ascontiguousarray(np.asarray(W_query, dtype=np.float32))
    W_key = np.ascontiguousarray(np.asarray(W_key, dtype=np.float32))
    W_value = np.ascontiguousarray(np.asarray(W_value, dtype=np.float32))

    nc = get_nc()
    in_maps = shard_inputs(x, W_query, W_key, W_value)
    last_err = None
    for _attempt in range(3):
        try:
            res = run_bass_kernel_spmd(nc, in_maps, core_ids=list(range(8)))
            return assemble_output(res.results)
        except Exception as e:  # transient device wedges seen on this fabric
            last_err = e
            import time as _time
            _time.sleep(2.0)
    raise last_err



# revision 5
# speedup vs baseline: 1.3439x; 1.3439x over previous
"""Multi-head causal attention Bass kernel for Trainium2, 8-core SPMD.

Problem: B=2, S=2048, D=1024, H=16, DH=64.
  q = x @ Wq; k = x @ Wk; v = x @ Wv  (per head h: 64-wide column slices)
  out = softmax(causal(q k^T / 8)) v

Sharding: core c -> batch b = c // 4, head group g = c % 4 (heads 4g..4g+3).
Each core gets x[b]^T and 256-wide W column slices (all bf16 on host),
computes 4 heads over the full sequence, returns y [2048, 256] f32 in
row-major [seq, (head, dh)] layout. Host assembly is a pure concat.

Per-core pipeline (all matmul operands bf16; psum f32):
  xT_ch[ch] [128, 8*512]   ch = s-chunk of 512; k-chunk kk at cols 512*kk
  w*_all    [128, 8*256]
  QT/KT[m][ch] [128, 512]  rows = 2 heads x 64 d of pair m, cols = seq
  V4[q]     [128, 4*260]   s-tiles 4q..; per head 64 V cols + 1 ones col
  scores^T per (pair, i-chunk c, j-tile jt): psum [128, 1024]
      (head half at col 512*half, only the causally valid Ni = 512-o
       i-columns are computed on diagonal tiles) -> exp via ACT
      (scale=1/8) -> e [128, 2*Ni] bf16; diagonal 128-col block masked
      in-place via gpsimd affine_select (keep i>=p, fill 0)
  ctx[i, e] via e-stationary matmuls: out[128 i, 65] += e_slice^T @
      [V_h | ones]; per (pair, head) accumulator psum [128, 4*65]
      (i-tile qq at col 65*qq, col 65*qq+64 = softmax denominator l)
  normalize: DVE reciprocal_approx_fast(l) [128,1] ->
      tensor_scalar_mul -> ctx_sb [128, 1024] f32 -> DMA per chunk
  Projections for chunk c+1 are interleaved into attention chunk c's
  PE stream so the tensor engine never idles while ACT runs exp.
"""

import sys

import numpy as np

try:
    import concourse.bass as bass  # noqa: F401
except ImportError:
    for _p in ("/opt/trn_rl_repo", "/root/.axon_site/_ro/trn_rl_repo"):
        if _p not in sys.path:
            sys.path.insert(0, _p)
    import concourse.bass as bass  # noqa: F401

from concourse import bacc
import concourse.mybir as mybir
import concourse.tile as tile

F32 = mybir.dt.float32
BF16 = mybir.dt.bfloat16

S = 2048          # sequence length
D = 1024          # model dim (contraction for projections)
HPC = 4           # heads per core
DH = 64           # head dim
NK = D // 128     # 8 contraction chunks
NCH = S // 512    # 4 s-chunks of 512
VW = DH + 1       # 65 cols per head in a V s-tile (V + ones)


def build_kernel(loop_n=0):
    nc = bacc.Bacc("TRN2", target_bir_lowering=False, debug=True)

    xT = nc.dram_tensor("xT", [D, S], BF16, kind="ExternalInput")
    wq = nc.dram_tensor("wq", [D, HPC * DH], BF16, kind="ExternalInput")
    wk = nc.dram_tensor("wk", [D, HPC * DH], BF16, kind="ExternalInput")
    wv = nc.dram_tensor("wv", [D, HPC * DH], BF16, kind="ExternalInput")
    y = nc.dram_tensor("y", [S, HPC * DH], F32, kind="ExternalOutput")

    with tile.TileContext(nc) as tc:
        from contextlib import ExitStack
        stk = ExitStack()
        loop = stk.enter_context(tc.For_i(0, loop_n, 1)) if loop_n else None
        with stk, (
            tc.tile_pool(name="persist", bufs=1)
        ) as pers, (
            tc.tile_pool(name="proj_ps", bufs=2, space="PSUM")
        ) as proj_ps, (
            tc.tile_pool(name="score_ps", bufs=2, space="PSUM")
        ) as score_ps, (
            tc.tile_pool(name="ctx_ps", bufs=2, space="PSUM")
        ) as ctx_ps_pool, (
            tc.tile_pool(name="esb", bufs=6)
        ) as esb_pool, (
            tc.tile_pool(name="norm", bufs=8)
        ) as norm_pool, (
            tc.tile_pool(name="ctxsb", bufs=2)
        ) as ctxsb_pool:
            # ---- persistent SBUF tiles -------------------------------------
            xT_ch = [
                pers.tile([128, NK * 512], BF16, tag=f"xTc{ch}", name=f"xTc{ch}")
                for ch in range(NCH)
            ]
            w_all = {
                wname: pers.tile([128, NK * HPC * DH], BF16, name=f"w_{wname}")
                for wname in ("q", "k", "v")
            }
            QT_sb = [
                [pers.tile([128, 512], BF16, tag=f"QT{m}c{ch}",
                           name=f"QT{m}c{ch}") for ch in range(NCH)]
                for m in range(2)
            ]
            KT_sb = [
                [pers.tile([128, 512], BF16, tag=f"KT{m}c{ch}",
                           name=f"KT{m}c{ch}") for ch in range(NCH)]
                for m in range(2)
            ]
            V4 = [
                pers.tile([128, 4 * HPC * VW], BF16, tag=f"V4_{q}",
                          name=f"V4_{q}")
                for q in range(4)
            ]

            def xs(ch, kk):      # xT chunk ch, k-chunk kk -> [128, 512]
                return xT_ch[ch][:, 512 * kk:512 * (kk + 1)]

            def ws(wname, kk):   # w k-chunk [128, 256]
                return w_all[wname][:, HPC * DH * kk:HPC * DH * (kk + 1)]

            def vs(t):           # V s-tile t -> [128, 260]
                q, r = t // 4, t % 4
                return V4[q][:, HPC * VW * r:HPC * VW * (r + 1)]

            # ---- input DMAs ------------------------------------------------
            # W on the SP HWDGE ring; xT s-chunks on the ACT ring so they
            # stream in parallel.
            for wname, wdram in (("q", wq), ("k", wk), ("v", wv)):
                nc.sync.dma_start(
                    out=w_all[wname].rearrange("p (k e) -> p k e", k=NK),
                    in_=wdram.rearrange("(k p) e -> p k e", k=NK),
                )
            xTr = xT.rearrange("(k p) (c s) -> p c k s", k=NK, c=NCH)
            for ch in range(NCH):
                nc.scalar.dma_start(
                    out=xT_ch[ch].rearrange("p (k s) -> p k s", k=NK),
                    in_=xTr[:, ch],
                )
            # ones columns of V (softmax denominator): one strided memset
            # per V4 group
            for q in range(4):
                nc.vector.memset(
                    V4[q].rearrange("p (t h c) -> p t h c", t=4, h=HPC)[
                        :, :, :, DH:DH + 1
                    ],
                    1.0,
                )

            # ---- projection pieces -----------------------------------------
            def emit_qk(ch, m, wname, dest):
                # Q^T/K^T: out[j, s] = sum_d W[d, j] * xT[d, s]
                ps = proj_ps.tile([128, 512], F32, tag="proj", name="ps_qk")
                for kk in range(NK):
                    nc.tensor.matmul(
                        ps[:],
                        ws(wname, kk)[:, 128 * m:128 * (m + 1)],
                        xs(ch, kk),
                        start=(kk == 0),
                        stop=(kk == NK - 1),
                    )
                nc.vector.tensor_copy(dest[m][ch][:], ps[:])

            def emit_v(t):
                # V: out[s, e] = sum_d xT[d, s] * Wv[d, e]
                ps = proj_ps.tile([128, HPC * DH], F32, tag="proj", name="ps_v")
                for kk in range(NK):
                    nc.tensor.matmul(
                        ps[:],
                        xs(t // 4, kk)[:, 128 * (t % 4):128 * (t % 4 + 1)],
                        ws("v", kk),
                        start=(kk == 0),
                        stop=(kk == NK - 1),
                    )
                nc.vector.tensor_copy(
                    vs(t).rearrange("p (h c) -> p h c", h=HPC)[:, :, 0:DH],
                    ps.rearrange("p (h c) -> p h c", h=HPC),
                )

            def proj_pieces(ch):
                pieces = []
                for m in range(2):
                    for wname, dest in (("q", QT_sb), ("k", KT_sb)):
                        pieces.append(
                            lambda ch=ch, m=m, w=wname, d=dest: emit_qk(ch, m, w, d))
                for t in range(4 * ch, 4 * ch + 4):
                    pieces.append(lambda t=t: emit_v(t))
                return pieces

            # Interleave plan: while attention chunk c runs, emit the
            # projection pieces listed here, spaced over c's steps.
            # Chunk 3's K/V projections are only needed from j-tile 12 on,
            # so they slide into attention chunk 3 itself; its Q must be
            # ready at step 0 and is emitted during chunk 2.
            p3 = proj_pieces(3)  # [q m0, k m0, q m1, k m1, v12, v13, v14, v15]
            plan = {
                0: proj_pieces(1),
                1: proj_pieces(2),
                2: [p3[0], p3[2]],                      # Q3 m0, m1
                3: [p3[1], p3[4], p3[5], p3[6], p3[7], p3[3]],  # K3 m0, V3, K3 m1
            }

            # ---- attention -------------------------------------------------
            def attn_step(c, pair, jt, ctx_ab):
                m = pair
                o = 128 * (jt - 4 * c) if jt >= 4 * c else 0
                ni = 512 - o
                ps = score_ps.tile([128, 1024], F32, tag="score", name="s_ps")
                for half in range(2):
                    off = half * 64
                    nc.tensor.matmul(
                        ps[:, 512 * half:512 * half + ni],
                        KT_sb[m][jt // 4][off:off + 64,
                                          128 * (jt % 4):128 * (jt % 4 + 1)],
                        QT_sb[m][c][off:off + 64, o:512],
                        start=True,
                        stop=True,
                        tile_position=(off, 0),
                    )
                e = esb_pool.tile([128, 2 * ni], BF16, tag="esb", name="e_sb")
                e3 = e.rearrange("p (h i) -> p h i", h=2)
                ps3 = ps.rearrange("p (h i) -> p h i", h=2)[:, :, 0:ni]
                nc.scalar.activation(
                    out=e3, in_=ps3,
                    func=mybir.ActivationFunctionType.Exp, scale=0.125,
                )
                if jt >= 4 * c:
                    # diagonal block: keep where i - p >= 0 within the
                    # first 128 valid i-columns
                    nc.gpsimd.affine_select(
                        e3[:, :, 0:128],
                        e3[:, :, 0:128],
                        pattern=[[0, 2], [1, 128]],
                        compare_op=mybir.AluOpType.is_ge,
                        fill=0.0,
                        base=0,
                        channel_multiplier=-1,
                    )
                # one psum accumulation group per bank: start zeroes the
                # whole 2KB zero-region, so only the first matmul into the
                # bank starts, later i-tile regions overwrite-on-first-touch
                for qq in range(4):
                    qg = 4 * c + qq
                    if jt > qg:
                        continue
                    i0 = 128 * qq - o
                    for head in range(2):
                        nc.tensor.matmul(
                            ctx_ab[head][:, VW * qq:VW * (qq + 1)],
                            e3[:, head, i0:i0 + 128],
                            vs(jt)[:, VW * (2 * pair + head):
                                   VW * (2 * pair + head + 1)],
                            start=(jt == 0 and qq == 0),
                            stop=(jt == 4 * c + 3 and qq == 3),
                        )

            def normalize(c, pair, head, qq, ctx_psum, ctx_sb_c):
                h = 2 * pair + head
                recip = norm_pool.tile([128, 1], F32, tag="recip", name="recip")
                nc.vector.reciprocal_approx_fast(
                    out=recip[:],
                    in_=ctx_psum[:, VW * qq + DH:VW * qq + DH + 1],
                )
                nc.vector.tensor_scalar_mul(
                    ctx_sb_c[:, 256 * qq + 64 * h:256 * qq + 64 * (h + 1)],
                    ctx_psum[:, VW * qq:VW * qq + DH],
                    recip[:],
                )

            # emit projections for chunk 0 up front
            for piece in proj_pieces(0):
                piece()

            for c in range(NCH):
                njt = 4 * (c + 1)
                steps = [(pair, jt) for pair in range(2) for jt in range(njt)]
                pieces = list(plan.get(c, []))
                n_steps = len(steps)
                n_pieces = len(pieces)
                emitted = 0
                ctx_sb_c = ctxsb_pool.tile([128, 4 * HPC * DH], F32,
                                           tag="ctxsb", name="ctx_sb")
                ctx_ab = {}
                for idx, (pair, jt) in enumerate(steps):
                    if jt == 0:
                        ctx_ab[pair] = [
                            ctx_ps_pool.tile([128, 4 * VW], F32, tag="ctx",
                                             name=f"ctx_ps{head}")
                            for head in range(2)
                        ]
                    attn_step(c, pair, jt, ctx_ab[pair])
                    # psum reads must wait for the bank's accumulation
                    # group to stop (last j-tile of the pair)
                    if jt == njt - 1:
                        for head in range(2):
                            for qq in range(4):
                                normalize(c, pair, head, qq,
                                          ctx_ab[pair][head], ctx_sb_c)
                    # paced projection interleave for a later chunk
                    if c == 3:
                        if idx < n_pieces:
                            pieces[idx]()
                            emitted += 1
                    else:
                        while (emitted < n_pieces
                               and emitted * n_steps <= idx * n_pieces):
                            pieces[emitted]()
                            emitted += 1
                while emitted < n_pieces:
                    pieces[emitted]()
                    emitted += 1
                for qq in range(4):
                    r0 = 512 * c + 128 * qq
                    nc.sync.dma_start(
                        out=y[r0:r0 + 128, :],
                        in_=ctx_sb_c[:, 256 * qq:256 * (qq + 1)],
                    )
    nc.compile()
    return nc


_CACHED = None


def get_nc():
    global _CACHED
    if _CACHED is None:
        _CACHED = build_kernel()
    return _CACHED


def shard_inputs(x, W_query, W_key, W_value):
    """Full inputs -> per-core input maps (bf16 on host)."""
    bf16 = mybir.dt.np(BF16)
    in_maps = []
    # one transpose per batch, shared by the 4 cores of that batch
    xT_by_batch = [np.ascontiguousarray(x[b].T).astype(bf16) for b in range(2)]
    wq_b = W_query.astype(bf16)
    wk_b = W_key.astype(bf16)
    wv_b = W_value.astype(bf16)
    for core in range(8):
        b, g = core // 4, core % 4
        sl = slice(256 * g, 256 * (g + 1))
        in_maps.append({
            "xT": xT_by_batch[b],
            "wq": np.ascontiguousarray(wq_b[:, sl]),
            "wk": np.ascontiguousarray(wk_b[:, sl]),
            "wv": np.ascontiguousarray(wv_b[:, sl]),
        })
    return in_maps


def assemble_output(results):
    """Per-core y [S, 256] -> full [2, S, 1024]."""
    out = np.empty((2, S, 1024), np.float32)
    for core in range(8):
        b, g = core // 4, core % 4
        out[b, :, 256 * g:256 * (g + 1)] = results[core]["y"]
    return out


def kernel(x, W_query, W_key, W_value):
    """Full inputs in, full output out; 8-core SPMD underneath."""
    from concourse.bass_utils import run_bass_kernel_spmd

    x = np.ascontiguousarray(np.asarray(x, dtype=np.float32))
    W_query = np.ascontiguousarray(np.asarray(W_query, dtype=np.float32))
    W_key = np.ascontiguousarray(np.asarray(W_key, dtype=np.float32))
    W_value = np.ascontiguousarray(np.asarray(W_value, dtype=np.float32))

    nc = get_nc()
    in_maps = shard_inputs(x, W_query, W_key, W_value)
    last_err = None
    for _attempt in range(3):
        try:
            res = run_bass_kernel_spmd(nc, in_maps, core_ids=list(range(8)))
            return assemble_output(res.results)
        except Exception as e:  # transient device wedges seen on this fabric
            last_err = e
            import time as _time
            _time.sleep(2.0)
    raise last_err
